# revision 1
# baseline (speedup 1.0000x reference)
"""Trainium2 Bass kernel for a GRU encoder-decoder (KLCPD generator).

Model (see reference):
  past_emb = relu(past @ W_emb + b_emb)            [T,B,E]
  fut_emb  = relu(future @ W_emb + b_emb)          [T,B,E]
  _, h_T   = GRU_enc(past_emb, h0=0)
  hidden   = h_T + noise
  ys, _    = GRU_dec(shift(fut_emb), h0=hidden)
  out      = ys @ W_out + b_out                    [T,B,D]

Sharding: data-parallel over batch B=1024 across 8 NeuronCores
(B_local=128); all weights replicated; no collectives.

Per-core kernel layout decisions:
  * All matmul inputs are bf16 (fp32 accumulation in PSUM).
  * The GRU hidden state is kept *transposed* in SBUF as
    hT[p, k*128 + b] = h[b, k*128 + p]  (k = H-chunk 0..3), so the
    elementwise gate math produces, with zero extra transposes, exactly
    the stationary operand needed by the next step's matmuls.
  * Gate pre-activations are accumulated in PSUM banks (r, z, hn double
    buffered = 6 banks; xn single buffered = 1 bank) in the same
    transposed layout; the input contribution x_emb @ W_ih of step t is
    accumulated into the same banks before the recurrent matmuls so it
    runs on the PE while step t-1's gate tail is still on ACT/DVE.
  * Embeddings STREAM through the loops instead of a separate phase:
    slot-batched input DMAs (16 timesteps per DMA; every DMA instruction
    costs ~625ns of engine-queue time, so batching matters), f32 PE
    transposes + matmul + relu through the one remaining PSUM bank, one
    group per 2 steps interleaved into the GRU loops as filler; the
    encoder loop starts ~10us into the kernel.
  * The output projection runs INSIDE the decoder loop: hT is the
    stationary operand so po[b, d] = (h @ W_out)[b, d] comes out of the
    PE directly (no transpose), sharing the embedding's PSUM bank;
    4 steps batch into one output DMA. No DRAM round-trip for ys.
  * Emission order is DMA priority order (first input slot -> w_emb ->
    enc weights on the scalar HWDGE queue -> remaining slots -> dec
    weights, with dec weight casts on the idle GpSimd engine).
Measured on HW (NTFF profile): 854us (previous phase-structured version)
-> 764us; TimelineSim predicts 733us.
"""

import os
from contextlib import ExitStack

import numpy as np

import concourse.bass as bass
import concourse.tile as tile
from concourse import bacc, bass_utils, masks, mybir
from concourse.tile_rust import add_dep_helper

T, B, D, E, H = 64, 1024, 128, 256, 512
NCORES = 8
BL = B // NCORES  # 128
H3 = 3 * H
P = 128

f32 = mybir.dt.float32
bf16 = mybir.dt.bfloat16
AF = mybir.ActivationFunctionType
OP = mybir.AluOpType


def _mm(nc, out, lhsT, rhs, start, stop):
    nc.tensor.matmul(out, lhsT, rhs, start=start, stop=stop, skip_group_check=True)


# Tunables (swept via TimelineSim, validated on HW).
CFG = {
    "tail_halves": 2,     # 1 = full-width gate ops, 2 = H-halved
    "w_on_gpsimd": False,  # offload w = z*h to the Pool engine
    "force_order": False,  # order half-1 DVE ops after h'-half0
    "emb_all_pre": False,  # emit all embedding groups before the enc loop
    "emb_lead": 1,         # steps between emb stage A and stage B
    "emb_e1_gpsimd": False,  # e1 relu on gpsimd instead of DVE
    "emb_cast_dve": False,   # xbf cast on DVE instead of gpsimd
    "emb_cadence": 2,        # steps between embedding-group stage As
    "tail_first_cols": 256,  # width of the first (chain-critical) tail slice
    "h_quarters": True,     # emit the final h op in 128-col quarters
    "po_direct_dma": False,  # accumulate 4 steps in one PSUM bank, DMA direct
}
for _k in list(CFG):
    _v = os.environ.get(f"KCFG_{_k.upper()}")
    if _v is not None:
        CFG[_k] = type(CFG[_k])(int(_v))


def build_module(zero_bias: bool, t_steps: int = T, dump_h: bool = False):
    """Builds the per-core Bass module. Returns the compiled nc."""
    nc = bacc.Bacc("TRN2", target_bir_lowering=False, debug=False)
    dbg_h = None
    if dump_h:
        dbg_h = nc.dram_tensor("dbg_h", [2, t_steps, P, H], bf16, kind="ExternalOutput").ap()

    past = nc.dram_tensor("past", [t_steps, BL, D], f32, kind="ExternalInput").ap()
    fut = nc.dram_tensor("fut", [t_steps, BL, D], f32, kind="ExternalInput").ap()
    noise = nc.dram_tensor("noise", [BL, H], f32, kind="ExternalInput").ap()
    w_emb = nc.dram_tensor("w_emb", [D, E], f32, kind="ExternalInput").ap()
    b_emb = nc.dram_tensor("b_emb", [1, E], f32, kind="ExternalInput").ap()
    wd = {}
    for g in ("enc", "dec"):
        wd[g, "ih"] = nc.dram_tensor(f"w_ih_{g}", [E, H3], f32, kind="ExternalInput").ap()
        wd[g, "hh"] = nc.dram_tensor(f"w_hh_{g}", [H, H3], f32, kind="ExternalInput").ap()
        wd[g, "bih"] = nc.dram_tensor(f"b_ih_{g}", [1, H3], f32, kind="ExternalInput").ap()
        wd[g, "bhh"] = nc.dram_tensor(f"b_hh_{g}", [1, H3], f32, kind="ExternalInput").ap()
    w_out = nc.dram_tensor("w_out", [H, D], f32, kind="ExternalInput").ap()
    b_out = nc.dram_tensor("b_out", [1, D], f32, kind="ExternalInput").ap()
    out = nc.dram_tensor("out", [t_steps, BL, D], f32, kind="ExternalOutput").ap()

    with tile.TileContext(nc, pool_alloc_mode="queue") as tc, ExitStack() as octx:
        wpool = octx.enter_context(tc.tile_pool(name="weights", bufs=1))

        # ---- constants -------------------------------------------------
        ident = wpool.tile([P, P], f32)
        masks.make_identity(nc, ident[:])
        ones_row = wpool.tile([1, 512], bf16)
        nc.gpsimd.memset(ones_row[:], 1.0)

        # ---- embedding precompute (streamed) ----------------------------
        # embT[g][e][p, t*BL + b] = relu(x[t] @ W_emb + b_emb)[b, e*128+p]
        # The input transpose uses the DMA XBAR (no PSUM, no PE); the
        # embedding matmuls all share ONE PSUM bank (pool `pgo`, reused
        # later by the decoder's output projection), leaving 7 banks for
        # the GRU gate accumulators so the encoder loop can start while
        # later embedding groups are still streaming in.
        embT = {g: [wpool.tile([P, t_steps * BL], bf16, name=f"embT_{g}_{e}", tag=f"embT_{g}_{e}")
                    for e in range(2)]
                for g in ("enc", "dec")}
        n_grp = t_steps // 4
        ep = octx.enter_context(tc.tile_pool(
            name="estage", bufs=int(os.environ.get("KCFG_EP_BUFS", 3 if zero_bias else 1))))
        pgo = octx.enter_context(tc.tile_pool(name="psum_out", bufs=1, space="PSUM"))

        # Slot-batched input loads: ONE DMA covers 4 embedding groups (16
        # timesteps), amortizing the ~625ns per-DMA engine-queue cost.
        GRP_PER_SLOT = int(os.environ.get("KCFG_GRP_PER_SLOT", 4))
        xs_slots = {}

        def emit_xs_slot(g, x_ap, si, split=False):
            lo = si * GRP_PER_SLOT * 4
            hi = min(lo + GRP_PER_SLOT * 4, t_steps)
            xs = ep.tile([P, (hi - lo) * P], f32, tag="xs")
            halves = ((lo, (lo + hi) // 2), ((lo + hi) // 2, hi)) if split else ((lo, hi),)
            for (a, b) in halves:
                nc.sync.dma_start(
                    xs[:, (a - lo) * P:(b - lo) * P].rearrange("p (i d) -> p i d", i=b - a),
                    x_ap[a:b].transpose([1, 0, 2]),
                )
            xs_slots[g, si] = xs

        def emit_emb_stage_a(g, x_ap, gi):
            """Transpose one 4-step group (PE transposes via a shared PSUM
            bank; input stays f32 until the PSUM->SBUF cast-copy).
            Returns a closure emitting stage B (matmul + relu)."""
            si, sub = divmod(gi, GRP_PER_SLOT)
            if (g, si) not in xs_slots:
                emit_xs_slot(g, x_ap, si,
                             split=os.environ.get("KCFG_SPLIT_ALL") == "1")
            xs = xs_slots[g, si]
            ptr = pgo.tile([P, 4 * P], f32, tag="po")
            for i in range(4):
                nc.tensor.transpose(ptr[:, i * P:(i + 1) * P],
                                    xs[:, (sub * 4 + i) * P:(sub * 4 + i + 1) * P],
                                    ident[:])
            xT = ep.tile([P, 4 * P], bf16, tag="xT")
            nc.scalar.copy(xT[:], ptr[:])

            def stage_b():
                for e in range(2):
                    pe_ = pgo.tile([P, 4 * P], f32, tag="po")
                    _mm(nc, pe_[:], wemb_bf[:, e * P:(e + 1) * P], xT[:],
                        start=True, stop=zero_bias)
                    if not zero_bias:
                        _mm(nc, pe_[:], bemb_bf[0:1, e * P:(e + 1) * P], ones_row[0:1, :],
                            start=False, stop=True)
                    dst = embT[g][e][:, gi * 4 * P:(gi + 1) * 4 * P]
                    if e == 0:
                        nc.scalar.activation(dst, pe_[:], AF.Relu)
                    elif CFG["emb_e1_gpsimd"]:
                        nc.gpsimd.tensor_scalar_max(dst, pe_[:], 0.0)
                    else:
                        nc.vector.tensor_scalar_max(dst, pe_[:], 0.0)

            return stage_b

        # Highest priority on the sync DMA ring: the first past-input
        # slot, so embedding group 0 (which gates the encoder loop) is
        # in flight before the big weight transfers.
        emit_xs_slot("enc", past, 0, split=True)

        # ---- weight preload + cast to bf16 -----------------------------
        # Emission order is priority order: everything the ENCODER loop
        # needs (w_emb, enc weights, past-input embeddings) is emitted
        # first; fut/dec data streams in behind it on the same queues and
        # is consumed much later (dec loop starts ~halfway through).
        # Input-tile DMAs ride the scalar (ACT) HWDGE queue so they do not
        # queue behind the big weight DMAs on the sync queue.
        whh = {}   # whh[g][k]: [128, H3]
        wih = {}   # wih[g][e]: [128, H3]
        biasx = {}  # [1, H3]  (b_ih + b_hh on r,z cols; b_ih on n cols)
        biashn = {}  # [1, 512] (b_hh n-part)
        stage_ctx = tc.tile_pool(name="wstage", bufs=2)
        stage = octx.enter_context(stage_ctx)


        wemb_bf = wpool.tile([P, E], bf16)
        st = stage.tile([P, E], f32, tag="s_emb")
        nc.sync.dma_start(st[:], w_emb[:, :])
        nc.vector.tensor_copy(wemb_bf[:], st[:])

        bemb_bf = None
        if not zero_bias:
            st = stage.tile([1, E], f32, tag="s_bemb")
            nc.sync.dma_start(st[:], b_emb[:, :])
            bemb_bf = wpool.tile([1, E], bf16)
            nc.vector.tensor_copy(bemb_bf[:], st[:])

        def load_gru_weights(g, cast_engine=None):
            # Encoder weights ride the scalar HWDGE queue so they do not
            # queue behind the input slots on the sync ring (and vice versa).
            dma_eng = nc.scalar if cast_engine is None else nc.sync
            wih[g] = []
            for e in range(2):
                t_ = wpool.tile([P, H3], bf16, tag=f"wih_{g}_{e}")
                st = stage.tile([P, H3], f32, tag="s_ih")
                dma_eng.dma_start(st[:], wd[g, "ih"][e * P:(e + 1) * P, :])
                if cast_engine is not None:
                    cast_engine.tensor_copy(t_[:], st[:])
                elif e % 2 == 0:
                    nc.vector.tensor_copy(t_[:], st[:])
                else:
                    nc.scalar.copy(t_[:], st[:])
                wih[g].append(t_)
            whh[g] = []
            for k in range(4):
                t_ = wpool.tile([P, H3], bf16, tag=f"whh_{g}_{k}")
                st = stage.tile([P, H3], f32, tag="s_hh")
                dma_eng.dma_start(st[:], wd[g, "hh"][k * P:(k + 1) * P, :])
                if cast_engine is None:
                    # alternate DVE/ACT so neither engine's queue carries
                    # all four casts during the loop ramp
                    if k % 2 == 0:
                        nc.vector.tensor_copy(t_[:], st[:])
                    else:
                        nc.scalar.copy(t_[:], st[:])
                else:
                    cast_engine.tensor_copy(t_[:], st[:])
                whh[g].append(t_)
            if not zero_bias:
                sih = stage.tile([1, H3], f32, tag="s_bih")
                shh = stage.tile([1, H3], f32, tag="s_bhh")
                nc.sync.dma_start(sih[:], wd[g, "bih"][:, :])
                nc.sync.dma_start(shh[:], wd[g, "bhh"][:, :])
                bx = wpool.tile([1, H3], bf16, tag=f"biasx_{g}")
                nc.vector.tensor_add(bx[:, 0:2 * H], sih[:, 0:2 * H], shh[:, 0:2 * H])
                nc.vector.tensor_copy(bx[:, 2 * H:H3], sih[:, 2 * H:H3])
                bh = wpool.tile([1, H], bf16, tag=f"biashn_{g}")
                nc.vector.tensor_copy(bh[:], shh[:, 2 * H:H3])
                biasx[g] = bx
                biashn[g] = bh

        load_gru_weights("enc")

        # noise, transposed on the PE via the shared PSUM bank (XBAR
        # DMA-transposes cost ~1.2us each on HW and would block the sync
        # queue ahead of the later input slots):
        # noiseT[p, k*128+b] = noise[b, k*128+p]
        noiseT = wpool.tile([P, H], bf16)
        st = stage.tile([P, H], f32, tag="s_noise")
        nc.sync.dma_start(st[:], noise[:, :])
        noise_st = st

        wout_bf = wpool.tile([P, 4 * P], bf16)  # col block k = W_out rows k
        st = stage.tile([P, 4 * P], f32, tag="s_out")
        for k in range(4):
            nc.sync.dma_start(st[:, k * P:(k + 1) * P], w_out[k * P:(k + 1) * P, :])
        nc.vector.tensor_copy(wout_bf[:], st[:])
        if not zero_bias:
            bout_bf = wpool.tile([1, D], bf16)
            st = stage.tile([1, D], f32, tag="s_bout")
            nc.sync.dma_start(st[:], b_out[:, :])
            nc.vector.tensor_copy(bout_bf[:], st[:])

        for _si in range(1, (n_grp + GRP_PER_SLOT - 1) // GRP_PER_SLOT):
            emit_xs_slot("enc", past, _si,
                         split=os.environ.get("KCFG_SPLIT_ALL") == "1")

        # First few past-input groups complete before the encoder loop;
        # the rest stream through the loop via the `extra` callback, one
        # two-stage group per two steps, filling idle engine windows.
        N_PRE = n_grp if CFG["emb_all_pre"] else int(os.environ.get("KCFG_N_PRE", 3))
        for gi in range(N_PRE):
            emit_emb_stage_a("enc", past, gi)()
        if CFG["emb_all_pre"]:
            for gi in range(n_grp):
                emit_emb_stage_a("dec", fut, gi)()

        emb_jobs = [("enc", past, gi) for gi in range(N_PRE, n_grp)]
        if not CFG["emb_all_pre"]:
            emb_jobs += [("dec", fut, gi) for gi in range(n_grp)]
        emb_jobs.reverse()  # consume via pop()
        emb_pending = []   # list of (due_step, stage_b)
        emb_clock = [0, -10]  # [global step counter, last stage-A step]

        def emb_filler(t):
            # Uses its own clock so the cadence spans both GRU loops: jobs
            # left over at the end of the encoder spill into the decoder's
            # (emptier) filler slots.
            tc_ = emb_clock[0]
            emb_clock[0] += 1
            while emb_pending and emb_pending[0][0] <= tc_:
                emb_pending.pop(0)[1]()
            if (not emb_pending and emb_jobs
                    and tc_ - emb_clock[1] >= CFG["emb_cadence"]):
                emb_clock[1] = tc_
                emb_pending.append((tc_ + CFG["emb_lead"], emit_emb_stage_a(*emb_jobs.pop())))


        # Noise transpose via the shared PSUM bank, sequenced after the
        # pre-loop embedding groups so it does not delay their bank chain.
        pnz = pgo.tile([P, H], f32, name="pnz", tag="po")
        for k in range(4):
            nc.tensor.transpose(pnz[:, k * P:(k + 1) * P], noise_st[:, k * P:(k + 1) * P], ident[:])
        nc.scalar.copy(noiseT[:], pnz[:])

        # Decoder weights: DMAs queue behind everything encoder-critical on
        # the sync ring; casts run on the otherwise-idle GpSimd engine so
        # they cannot delay the encoder loop's ACT/DVE tails.
        load_gru_weights("dec", cast_engine=nc.gpsimd)



        # ---- GRU loops --------------------------------------------------
        def gru_loop(g, is_dec, hT0, sb, pg, pgx, pgo, extra=None):
            """Runs t_steps of GRU g. hT0 = initial transposed state (or None).
            Returns final hT tile."""
            hT_prev = hT0

            def emit_xw(t):
                """Allocate step t's PSUM banks and emit its input-projection
                matmuls. Returns the bank state for the recurrent matmuls and
                gate tail."""
                have_x = (not is_dec) or t > 0
                have_h = t > 0 or hT0 is not None
                have_xn = have_x or not zero_bias
                pr = pg.tile([P, H], f32, name="pr", tag="pr")
                pz = pg.tile([P, H], f32, name="pz", tag="pz")
                pxn = pgx.tile([P, H], f32, name="pxn", tag="pxn") if have_xn else None
                phn = pg.tile([P, H], f32, name="phn", tag="phn") if have_h else None

                # One start=True per PSUM bank per step (the hardware's
                # pending-zero covers the whole 2KB bank); one stop on the
                # bank's last matmul. Track per-bank emitted/total counts.
                nbias = 0 if zero_bias else 1
                nxw = (2 if have_x else 0) + nbias
                nhw = 4 if have_h else 0
                totals = {id(pr): 4 * (nxw + nhw), id(pz): 4 * (nxw + nhw)}
                if pxn is not None:
                    totals[id(pxn)] = 4 * nxw
                if phn is not None:
                    totals[id(phn)] = 4 * (4 + nbias)
                emitted = {k: 0 for k in totals}

                def emit(bank, sl, lhsT, rhs):
                    emitted[id(bank)] += 1
                    _mm(nc, sl, lhsT, rhs,
                        start=emitted[id(bank)] == 1,
                        stop=emitted[id(bank)] == totals[id(bank)])

                tcol = (t - 1) if is_dec else t
                lx = ([embT[g][e][:, tcol * BL:(tcol + 1) * BL] for e in range(2)]
                      if have_x else None)
                for bank, lo in ((pr, 0), (pz, H), (pxn, 2 * H)):
                    if bank is None:
                        continue
                    for m in range(4):
                        sl = bank[:, m * P:(m + 1) * P]
                        if not zero_bias:
                            emit(bank, sl, biasx[g][0:1, lo + m * P:lo + (m + 1) * P],
                                 ones_row[0:1, 0:P])
                        if have_x:
                            for e in range(2):
                                emit(bank, sl, wih[g][e][:, lo + m * P:lo + (m + 1) * P], lx[e])
                return pr, pz, pxn, phn, emit

            po_buf = [None]

            def emit_po(t, hT):
                """Output projection of decoder step t: out[t] = h @ W_out
                (+ b_out). hT is the transposed state, so using it as the
                stationary operand yields po[b, d] directly — no transpose.
                Results are batched 4 steps per output DMA."""
                sub = t % 4
                if CFG["po_direct_dma"]:
                    # Four steps accumulate into quarters of ONE bank (the
                    # step-0 start=True pending-zeroes the whole bank), then
                    # one wide PSUM->SBUF copy + one DMA.
                    if sub == 0:
                        po_buf[0] = pgo.tile([P, 512], f32, name="po", tag="po")
                    po = po_buf[0]
                    sl = po[:, sub * P:(sub + 1) * P]
                    if not zero_bias:
                        _mm(nc, sl, ones_row[0:1, 0:P], bout_bf[0:1, :],
                            start=sub == 0, stop=False)
                    for k in range(4):
                        _mm(nc, sl, hT[:, k * P:(k + 1) * P], wout_bf[:, k * P:(k + 1) * P],
                            start=zero_bias and sub == 0 and k == 0, stop=k == 3)
                    if sub == 3:
                        outf = sb.tile([P, 4 * P], f32, name="outf", tag="outf")
                        nc.scalar.copy(outf[:], po[:])
                        nc.sync.dma_start(
                            out[t - 3:t + 1].transpose([1, 0, 2]),
                            outf[:].rearrange("p (i d) -> p i d", i=4),
                        )
                    return
                po = pgo.tile([P, 512], f32, name="po", tag="po")
                sl = po[:, 0:P]
                if not zero_bias:
                    _mm(nc, sl, ones_row[0:1, 0:P], bout_bf[0:1, :],
                        start=True, stop=False)
                for k in range(4):
                    _mm(nc, sl, hT[:, k * P:(k + 1) * P], wout_bf[:, k * P:(k + 1) * P],
                        start=zero_bias and k == 0, stop=k == 3)
                if po_buf[0] is None:
                    po_buf[0] = sb.tile([P, 4 * P], f32, name="outf", tag="outf")
                nc.scalar.copy(po_buf[0][:, sub * P:(sub + 1) * P], sl)
                if sub == 3:
                    nc.sync.dma_start(
                        out[t - 3:t + 1].transpose([1, 0, 2]),
                        po_buf[0][:].rearrange("p (i d) -> p i d", i=4),
                    )
                    po_buf[0] = None

            prev_h_out = None
            state = emit_xw(0)
            for t in range(t_steps):
                have_x = (not is_dec) or t > 0
                have_h = hT_prev is not None
                have_xn = have_x or not zero_bias
                pr, pz, pxn, phn, emit = state

                # -- hW(t): recurrent matmuls. Batch all k∈{0,1} first (they
                # depend only on the first half of h'(t-1), which the tail
                # produces early), then k∈{2,3}. Step t+1's input-projection
                # matmuls are emitted BETWEEN the batches so the PE fills the
                # wait for h'(t-1)'s second half with useful work. Note:
                # matmuls within one PSUM bank execute as an emission-ordered
                # chain, so h1-gated MMs must never precede h0-ready MMs of
                # the same bank (m01-first orderings lose ~50us to this). ---
                def hw_batch(kpair):
                    # bank-major: all r slices first (both gate halves), then
                    # hn, then z — the chain-critical sigmoid(r) of each half
                    # unblocks as early as possible.
                    for bank, lo in ((pr, 0), (phn, 2 * H), (pz, H)):
                        for m in range(4):
                            sl = bank[:, m * P:(m + 1) * P]
                            if bank is phn and not zero_bias and kpair[0] == 0:
                                emit(bank, sl, biashn[g][0:1, m * P:(m + 1) * P],
                                     ones_row[0:1, 0:P])
                            for k in kpair:
                                emit(bank, sl, whh[g][k][:, lo + m * P:lo + (m + 1) * P],
                                     hT_prev[:, k * P:(k + 1) * P])

                if have_h:
                    hw_batch((0, 1))
                if prev_h_out is not None:
                    # Previous step's output projection: emitted after the
                    # k01 recurrent batch so it fills this step's tail window
                    # on the PE instead of delaying the critical-path matmuls.
                    emit_po(t - 1, prev_h_out)
                    prev_h_out = None
                if t + 1 < t_steps:
                    state = emit_xw(t + 1)
                if have_h:
                    hw_batch((2, 3))

                # -- gate math (transposed layout), chunked in two halves --
                HH = H // 2
                r_t = sb.tile([P, H], bf16, name="r_t", tag="r")
                z_t = sb.tile([P, H], bf16, name="z_t", tag="z")
                n_t = sb.tile([P, H], bf16, tag="n")
                p_t = sb.tile([P, H], bf16, tag="p")
                h_new = sb.tile([P, H], bf16, tag="h")
                if have_h:
                    t1 = sb.tile([P, H], bf16, tag="t1")
                    t2 = sb.tile([P, H], bf16, name="t2", tag="t2") if have_xn else t1
                    w_t = sb.tile([P, H], bf16, tag="w")
                h0_done = None
                fc = CFG["tail_first_cols"]
                tail_slices = ([slice(0, fc), slice(fc, H)] if CFG["tail_halves"] == 2
                               else [slice(0, H)])
                for half, hs in enumerate(tail_slices):
                    half_ops = []
                    nc.scalar.activation(r_t[:, hs], pr[:, hs], AF.Sigmoid)
                    nc.scalar.activation(z_t[:, hs], pz[:, hs], AF.Sigmoid)
                    if have_h:
                        half_ops.append(nc.vector.tensor_mul(t1[:, hs], r_t[:, hs], phn[:, hs]))
                        if have_xn:
                            half_ops.append(nc.vector.tensor_add(t2[:, hs], t1[:, hs], pxn[:, hs]))
                        n_src = t2
                    else:
                        n_src = pxn
                    nc.scalar.activation(n_t[:, hs], n_src[:, hs], AF.Tanh)
                    if have_h:
                        weng = nc.gpsimd if CFG["w_on_gpsimd"] else nc.vector
                        half_ops.append(weng.tensor_mul(w_t[:, hs], z_t[:, hs], hT_prev[:, hs]))
                        half_ops.append(nc.vector.scalar_tensor_tensor(
                            p_t[:, hs], z_t[:, hs], 1.0, n_t[:, hs], OP.subtract, OP.mult))
                        if CFG["h_quarters"]:
                            # 128-col writes so the next step's k-chunk
                            # matmuls unblock per-quarter (region deps).
                            for q0 in range(hs.start, hs.stop, P):
                                qs = slice(q0, q0 + P)
                                hdone = nc.vector.tensor_sub(h_new[:, qs], w_t[:, qs], p_t[:, qs])
                        else:
                            hdone = nc.vector.tensor_sub(h_new[:, hs], w_t[:, hs], p_t[:, hs])
                    else:
                        half_ops.append(nc.vector.scalar_tensor_tensor(
                            p_t[:, hs], z_t[:, hs], 1.0, n_t[:, hs], OP.subtract, OP.mult))
                        hdone = nc.vector.tensor_scalar_mul(h_new[:, hs], p_t[:, hs], -1.0)
                    if half == 0:
                        h0_done = hdone
                    elif CFG["force_order"]:
                        # Keep the DVE focused on finishing h'-half0 before it
                        # starts half-1 work: the next step's k01 matmuls are
                        # gated on half 0.
                        for op in half_ops:
                            add_dep_helper(h0_done.ins, op.ins, sync=False,
                                           reason="finish h half0 first")
                hT_prev = h_new
                if extra is not None:
                    # After the tail ops so the filler's ACT/DVE work queues
                    # BEHIND the step's critical sigmoid/tanh chain.
                    extra(t)
                if dbg_h is not None:
                    nc.sync.dma_start(dbg_h[1 if is_dec else 0, t], h_new[:])
                if is_dec:
                    prev_h_out = h_new
            if prev_h_out is not None:
                emit_po(t_steps - 1, prev_h_out)
            return hT_prev

        with tc.tile_pool(name="gru_sb", bufs=int(os.environ.get("KCFG_SB_BUFS", 3 if zero_bias else 2))) as sb, \
             tc.tile_pool(name="psum_g", bufs=2, space="PSUM") as pg, \
             tc.tile_pool(name="psum_gx", bufs=1, space="PSUM") as pgx:
            hT_enc = gru_loop("enc", False, None, sb, pg, pgx, pgo, extra=emb_filler)
            hid = sb.tile([P, H], bf16, tag="h")
            for _q in range(0, H, P):
                nc.vector.tensor_add(hid[:, _q:_q + P], hT_enc[:, _q:_q + P],
                                     noiseT[:, _q:_q + P])
            gru_loop("dec", True, hid, sb, pg, pgx, pgo, extra=emb_filler)

    nc.compile()
    return nc


_CACHE = {}


def _get_module(zero_bias: bool):
    key = zero_bias
    if key not in _CACHE:
        _CACHE[key] = build_module(zero_bias)
    return _CACHE[key]


def kernel(past_input, future_input, noise,
           W_emb, b_emb,
           W_ih_enc, W_hh_enc, b_ih_enc, b_hh_enc,
           W_ih_dec, W_hh_dec, b_ih_dec, b_hh_dec,
           W_out, b_out):
    f = np.float32
    past_input = np.asarray(past_input, f)
    future_input = np.asarray(future_input, f)
    noise = np.asarray(noise, f)
    zero_bias = not any(
        np.any(np.asarray(b)) for b in (b_emb, b_ih_enc, b_hh_enc, b_ih_dec, b_hh_dec, b_out)
    )
    nc = _get_module(zero_bias)

    shared = {
        "w_emb": np.asarray(W_emb, f),
        "b_emb": np.asarray(b_emb, f).reshape(1, E),
        "w_ih_enc": np.asarray(W_ih_enc, f), "w_hh_enc": np.asarray(W_hh_enc, f),
        "b_ih_enc": np.asarray(b_ih_enc, f).reshape(1, H3),
        "b_hh_enc": np.asarray(b_hh_enc, f).reshape(1, H3),
        "w_ih_dec": np.asarray(W_ih_dec, f), "w_hh_dec": np.asarray(W_hh_dec, f),
        "b_ih_dec": np.asarray(b_ih_dec, f).reshape(1, H3),
        "b_hh_dec": np.asarray(b_hh_dec, f).reshape(1, H3),
        "w_out": np.asarray(W_out, f),
        "b_out": np.asarray(b_out, f).reshape(1, D),
    }
    in_maps = []
    for c in range(NCORES):
        sl = slice(c * BL, (c + 1) * BL)
        m = dict(shared)
        m["past"] = np.ascontiguousarray(past_input[:, sl, :])
        m["fut"] = np.ascontiguousarray(future_input[:, sl, :])
        m["noise"] = np.ascontiguousarray(noise[sl, :])
        in_maps.append(m)

    res = bass_utils.run_bass_kernel_spmd(nc, in_maps, core_ids=list(range(NCORES)))
    return np.concatenate([r["out"] for r in res.results], axis=1)

